# revision 1
# baseline (speedup 1.0000x reference)
"""Householder reflection kernel for Trainium2 (8 NeuronCores, data-parallel).

Computes: v_new = v @ W.T + b
          z_new = z - 2 * v_new * (v_new . z) / ||v_new||^2

Full inputs: z [524288, 128] f32, v [524288, 128] f32, W [128, 128] f32, b [128] f32.
Sharding: batch split 8 ways; W/b replicated.

Strategy (fp16 data movement, fp32 accumulation):
  - Host: cast z to fp16; build vt = v.T in fp16 with rows permuted so that
    both the z/out DMAs and the vt DMA are 4KB-contiguous per partition,
    and each matmul's stationary operand is a [feat, rows] chunk of vt
    (no on-chip transposes needed at all).
    Row mapping: global row r = g*2048 + p*16 + k  <->  (group g, partition p,
    chunk k); vt column order is g*2048 + k*128 + p.
  - PE: per 128-row chunk, psum[:,k,:] = vt_chunk.T @ W.T (fp16 in, fp32 acc),
    bias added via a rank-1 ones*b4 matmul per PSUM bank (4 chunks at once).
  - ACT: copy psum -> vn (fp16 sbuf) and Square psum -> sq (fp16 sbuf).
  - DVE: tree-reduce sq -> norm [128,16] f32; prod = vn*z (2x fp16 mode) and
    tree-reduce -> dot; s = -2*dot*recip(norm).
  - Pool (gpsimd): per-chunk fused z_new = vn*s + z.
"""

import sys

if "/opt/trn_rl_repo" not in sys.path:
    sys.path.insert(0, "/opt/trn_rl_repo")

import numpy as np

B = 524288
D = 128
NCORES = 8
ROWS_PER_CORE = B // NCORES          # 65536
P = 128                              # rows per chunk (psum partitions)
K = 16                               # chunks per group
ROWS_PER_GROUP = K * P               # 2048
GROUPS = ROWS_PER_CORE // ROWS_PER_GROUP  # 32
GT = B // ROWS_PER_GROUP             # 256 global groups

TRACE = False
TRACE_KW = {}
LAST = None

_compiled = None


def _build():
    import concourse.bacc as bacc
    import concourse.tile as tile
    from concourse import mybir

    nc = bacc.Bacc("TRN2")
    f16 = mybir.dt.float16
    f32 = mybir.dt.float32
    Alu = mybir.AluOpType
    Act = mybir.ActivationFunctionType

    z_d = nc.dram_tensor("z", [ROWS_PER_CORE, D], f16, kind="ExternalInput")
    vt_d = nc.dram_tensor("vt", [D, ROWS_PER_CORE], f16, kind="ExternalInput")
    wt_d = nc.dram_tensor("wt", [D, D], f16, kind="ExternalInput")
    b4_d = nc.dram_tensor("b4", [1, 4 * D], f16, kind="ExternalInput")
    out_d = nc.dram_tensor("z_new", [ROWS_PER_CORE, D], f16, kind="ExternalOutput")

    # DRAM views; all give >=4KB-contiguous per-partition DMA segments.
    # Loads are paired two groups per DMA to halve DGE/semaphore overhead.
    zv2 = z_d.rearrange("(g t p k) f -> g p t k f", t=2, p=P, k=K)
    ov = out_d.rearrange("(g p k) f -> g p k f", p=P, k=K)
    vv2 = vt_d.rearrange("f (g t k p) -> g f t k p", t=2, k=K, p=P)

    with tile.TileContext(nc) as tc:
        from contextlib import ExitStack

        with ExitStack() as ctx:
            singles = ctx.enter_context(tc.tile_pool(name="singles", bufs=1))
            inpool = ctx.enter_context(tc.tile_pool(name="inp", bufs=3))
            vnpool = ctx.enter_context(tc.tile_pool(name="vn", bufs=5))
            sqpool = ctx.enter_context(tc.tile_pool(name="sq", bufs=4))
            hpool = ctx.enter_context(tc.tile_pool(name="h", bufs=4))
            smpool = ctx.enter_context(tc.tile_pool(name="sm", bufs=6))
            znpool = ctx.enter_context(tc.tile_pool(name="zn", bufs=4))
            ppool = ctx.enter_context(tc.tile_pool(name="ps", bufs=2, space="PSUM"))

            wt_sb = singles.tile([D, D], f16)
            nc.sync.dma_start(out=wt_sb, in_=wt_d.ap())
            b4_sb = singles.tile([1, 4 * D], f16)
            nc.sync.dma_start(out=b4_sb, in_=b4_d.ap())
            ones1 = singles.tile([1, D], f16)
            nc.vector.memset(ones1, 1.0)

            prev = None
            for g in range(GROUPS):
                if g % 2 == 0:
                    vt2 = inpool.tile([P, 2, K, P], f16, tag="vt")
                    z2 = inpool.tile([P, 2, K, D], f16, tag="z")
                    nc.sync.dma_start(out=vt2, in_=vv2[g // 2])
                    nc.sync.dma_start(out=z2, in_=zv2[g // 2])
                vt_t = vt2[:, g % 2]
                z_t = z2[:, g % 2]

                # Half-group PSUM tiles (2 banks, 2 bufs per tag): ACT drains
                # each half while PE fills the next, shortening the
                # PE(g+1) <- COPY(g) PSUM-reuse stall.
                vn_t = vnpool.tile([P, K, D], f16, tag="vn")
                H = K // 2
                for h, ptag in ((0, "psa"), (1, "psb")):
                    ps = ppool.tile([P, H, D], f32, tag=ptag)
                    for bb in range(H // 4):
                        nc.tensor.matmul(
                            ps[:, 4 * bb : 4 * bb + 4, :],
                            lhsT=ones1,
                            rhs=b4_sb,
                            start=True,
                            stop=False,
                            skip_group_check=True,
                        )
                        for kk in range(4 * bb, 4 * bb + 4):
                            k = h * H + kk
                            nc.tensor.matmul(
                                ps[:, kk, :],
                                lhsT=vt_t[:, k, :],
                                rhs=wt_sb,
                                start=False,
                                stop=True,
                                skip_group_check=True,
                            )
                    nc.scalar.copy(out=vn_t[:, h * H : (h + 1) * H, :], in_=ps)
                # Square from the fp16 copy, not PSUM: PSUM's only reader is
                # then the copy, so PE(g+1) can reuse the PSUM slot ~2us
                # earlier (its WAR binds to the copy, not the square).
                sq_t = sqpool.tile([P, K, D], f16, tag="sq")
                nc.scalar.activation(out=sq_t, in_=vn_t, func=Act.Square)

                # norm = rowsum(vn^2), dot = rowsum(vn*z): tree halvings
                # (fp16, DVE 2x) into one combined tile, then one flat reduce.
                # pr first (only needs the ACT copy), hn1 next (needs ACT sq).
                pr_t = sqpool.tile([P, K, D], f16, tag="pr")
                nc.vector.tensor_tensor(out=pr_t, in0=vn_t, in1=z_t, op=Alu.mult)
                hd1 = hpool.tile([P, K, 64], f16, tag="hd1")
                nc.vector.tensor_tensor(
                    out=hd1, in0=pr_t[:, :, 0:64], in1=pr_t[:, :, 64:128], op=Alu.add
                )
                hn1 = hpool.tile([P, K, 64], f16, tag="hn1")
                nc.vector.tensor_tensor(
                    out=hn1, in0=sq_t[:, :, 0:64], in1=sq_t[:, :, 64:128], op=Alu.add
                )
                h2 = hpool.tile([P, 2 * K, 32], f16, tag="h2")
                nc.vector.tensor_tensor(
                    out=h2[:, 0:K, :], in0=hn1[:, :, 0:32], in1=hn1[:, :, 32:64], op=Alu.add
                )
                nc.vector.tensor_tensor(
                    out=h2[:, K : 2 * K, :], in0=hd1[:, :, 0:32], in1=hd1[:, :, 32:64], op=Alu.add
                )
                nd_t = smpool.tile([P, 2 * K], f32, tag="nd")
                nc.vector.tensor_reduce(
                    out=nd_t, in_=h2, axis=mybir.AxisListType.X, op=Alu.add
                )

                rn_t = smpool.tile([P, K], f32, tag="rn")
                nc.vector.reciprocal(out=rn_t, in_=nd_t[:, 0:K])
                s_t = smpool.tile([P, K], f32, tag="s")
                nc.vector.scalar_tensor_tensor(
                    out=s_t,
                    in0=nd_t[:, K : 2 * K],
                    scalar=-2.0,
                    in1=rn_t,
                    op0=Alu.mult,
                    op1=Alu.mult,
                )

                # t = vn * s via one Pool tensor_tensor with a stride-0
                # broadcast of s along features.
                t_t = vnpool.tile([P, K, D], f16, tag="t")
                s_bc = s_t.unsqueeze(2).broadcast_to([P, K, D])
                nc.gpsimd.tensor_tensor(out=t_t, in0=vn_t, in1=s_bc, op=Alu.mult)

                # Software-pipelined tail: while Pool computes t(g), DVE
                # finishes z_new(g-1) = t(g-1) + z(g-1) (fp16 2x) so the
                # in-order DVE queue never stalls on the Pool round-trip.
                if prev is not None:
                    pt, pz, pg = prev
                    zn_t = znpool.tile([P, K, D], f16, tag="zn")
                    nc.vector.tensor_tensor(out=zn_t, in0=pt, in1=pz, op=Alu.add)
                    nc.scalar.dma_start(out=ov[pg], in_=zn_t)
                prev = (t_t, z_t, g)

            pt, pz, pg = prev
            zn_t = znpool.tile([P, K, D], f16, tag="zn")
            nc.vector.tensor_tensor(out=zn_t, in0=pt, in1=pz, op=Alu.add)
            nc.scalar.dma_start(out=ov[pg], in_=zn_t)

    nc.compile()
    return nc


def _get_compiled():
    global _compiled
    if _compiled is None:
        _compiled = _build()
    return _compiled


def kernel(z, v, W, b):
    from concourse.bass_utils import run_bass_kernel_spmd

    nc = _get_compiled()

    z16 = np.ascontiguousarray(np.asarray(z), dtype=np.float16)
    v16 = np.asarray(v, dtype=np.float16)
    # vt[f, g*2048 + k*128 + p] = v[g*2048 + p*16 + k, f]
    vt = np.ascontiguousarray(
        v16.reshape(GT, P, K, D).transpose(3, 0, 2, 1).reshape(D, B)
    )
    wt = np.ascontiguousarray(np.asarray(W, dtype=np.float16).T)
    b4 = np.ascontiguousarray(
        np.tile(np.asarray(b, dtype=np.float16).reshape(1, D), (1, 4))
    )

    in_maps = []
    for c in range(NCORES):
        sl = slice(c * ROWS_PER_CORE, (c + 1) * ROWS_PER_CORE)
        in_maps.append(
            {
                "z": z16[sl],
                "vt": np.ascontiguousarray(vt[:, sl]),
                "wt": wt,
                "b4": b4,
            }
        )

    res = run_bass_kernel_spmd(
        nc, in_maps, core_ids=list(range(NCORES)), trace=TRACE, **TRACE_KW
    )
    global LAST
    LAST = res
    out16 = np.concatenate([res.results[c]["z_new"] for c in range(NCORES)], axis=0)
    return out16.astype(np.float32)



# revision 2
# speedup vs baseline: 1.0261x; 1.0261x over previous
"""Householder reflection kernel for Trainium2 (8 NeuronCores, data-parallel).

Computes: v_new = v @ W.T + b
          z_new = z - 2 * v_new * (v_new . z) / ||v_new||^2

Full inputs: z [524288, 128] f32, v [524288, 128] f32, W [128, 128] f32, b [128] f32.
Sharding: batch split 8 ways; W/b replicated.

Strategy (fp16 data movement, fp32 accumulation):
  - Host: cast z to fp16; build vt = v.T in fp16 with rows permuted so that
    both the z/out DMAs and the vt DMA are 4KB-contiguous per partition,
    and each matmul's stationary operand is a [feat, rows] chunk of vt
    (no on-chip transposes needed at all).
    Row mapping: global row r = g*2048 + p*16 + k  <->  (group g, partition p,
    chunk k); vt column order is g*2048 + k*128 + p.
  - PE: per 128-row chunk, psum[:,k,:] = vt_chunk.T @ W.T (fp16 in, fp32 acc),
    bias added via a rank-1 ones*b4 matmul per PSUM bank (4 chunks at once).
  - ACT: copy psum -> vn (fp16 sbuf) and Square psum -> sq (fp16 sbuf).
  - DVE: tree-reduce sq -> norm [128,16] f32; prod = vn*z (2x fp16 mode) and
    tree-reduce -> dot; s = -2*dot*recip(norm).
  - Pool (gpsimd): per-chunk fused z_new = vn*s + z.
"""

import sys

if "/opt/trn_rl_repo" not in sys.path:
    sys.path.insert(0, "/opt/trn_rl_repo")

import numpy as np

B = 524288
D = 128
NCORES = 8
ROWS_PER_CORE = B // NCORES          # 65536
P = 128                              # rows per chunk (psum partitions)
K = 16                               # chunks per group
ROWS_PER_GROUP = K * P               # 2048
GROUPS = ROWS_PER_CORE // ROWS_PER_GROUP  # 32
GT = B // ROWS_PER_GROUP             # 256 global groups

TRACE = False
TRACE_KW = {}
LAST = None

_compiled = None


def _build():
    import concourse.bacc as bacc
    import concourse.tile as tile
    from concourse import mybir

    nc = bacc.Bacc("TRN2")
    f16 = mybir.dt.float16
    f32 = mybir.dt.float32
    Alu = mybir.AluOpType
    Act = mybir.ActivationFunctionType

    z_d = nc.dram_tensor("z", [ROWS_PER_CORE, D], f16, kind="ExternalInput")
    vt_d = nc.dram_tensor("vt", [D, ROWS_PER_CORE], f16, kind="ExternalInput")
    wt_d = nc.dram_tensor("wt", [D, D], f16, kind="ExternalInput")
    b4_d = nc.dram_tensor("b4", [1, 4 * D], f16, kind="ExternalInput")
    out_d = nc.dram_tensor("z_new", [ROWS_PER_CORE, D], f16, kind="ExternalOutput")

    # DRAM views; all give >=4KB-contiguous per-partition DMA segments.
    # Loads are paired two groups per DMA to halve DGE/semaphore overhead.
    zv2 = z_d.rearrange("(g t p k) f -> g p t k f", t=2, p=P, k=K)
    ov = out_d.rearrange("(g p k) f -> g p k f", p=P, k=K)
    vv2 = vt_d.rearrange("f (g t k p) -> g f t k p", t=2, k=K, p=P)

    with tile.TileContext(nc) as tc:
        from contextlib import ExitStack

        with ExitStack() as ctx:
            singles = ctx.enter_context(tc.tile_pool(name="singles", bufs=1))
            inpool = ctx.enter_context(tc.tile_pool(name="inp", bufs=3))
            vnpool = ctx.enter_context(tc.tile_pool(name="vn", bufs=5))
            sqpool = ctx.enter_context(tc.tile_pool(name="sq", bufs=4))
            hpool = ctx.enter_context(tc.tile_pool(name="h", bufs=4))
            smpool = ctx.enter_context(tc.tile_pool(name="sm", bufs=6))
            znpool = ctx.enter_context(tc.tile_pool(name="zn", bufs=4))
            ppool = ctx.enter_context(tc.tile_pool(name="ps", bufs=2, space="PSUM"))

            wt_sb = singles.tile([D, D], f16)
            nc.sync.dma_start(out=wt_sb, in_=wt_d.ap())
            b4_sb = singles.tile([1, 4 * D], f16)
            nc.sync.dma_start(out=b4_sb, in_=b4_d.ap())
            ones1 = singles.tile([1, D], f16)
            nc.vector.memset(ones1, 1.0)

            prev = None
            for g in range(GROUPS):
                if g % 2 == 0:
                    vt2 = inpool.tile([P, 2, K, P], f16, tag="vt")
                    z2 = inpool.tile([P, 2, K, D], f16, tag="z")
                    nc.sync.dma_start(out=vt2, in_=vv2[g // 2])
                    nc.sync.dma_start(out=z2, in_=zv2[g // 2])
                vt_t = vt2[:, g % 2]
                z_t = z2[:, g % 2]

                # Half-group PSUM tiles (2 banks, 2 bufs per tag): ACT drains
                # each half while PE fills the next, shortening the
                # PE(g+1) <- COPY(g) PSUM-reuse stall.
                vn_t = vnpool.tile([P, K, D], f16, tag="vn")
                H = K // 2
                for h, ptag in ((0, "psa"), (1, "psb")):
                    ps = ppool.tile([P, H, D], f32, tag=ptag)
                    for bb in range(H // 4):
                        nc.tensor.matmul(
                            ps[:, 4 * bb : 4 * bb + 4, :],
                            lhsT=ones1,
                            rhs=b4_sb,
                            start=True,
                            stop=False,
                            skip_group_check=True,
                        )
                        for kk in range(4 * bb, 4 * bb + 4):
                            k = h * H + kk
                            nc.tensor.matmul(
                                ps[:, kk, :],
                                lhsT=vt_t[:, k, :],
                                rhs=wt_sb,
                                start=False,
                                stop=True,
                                skip_group_check=True,
                            )
                    nc.scalar.copy(out=vn_t[:, h * H : (h + 1) * H, :], in_=ps)

                # Combined tile: [:, 0:K] = pr = vn*z (DVE), [:, K:2K] = vn^2
                # (ACT Square). One shared tree then reduces dot and norm
                # together: 3 DVE ops instead of 5 (per-op overhead is the
                # dominant cost at these sizes).
                cmb = sqpool.tile([P, 2 * K, D], f16, tag="cmb")
                nc.vector.tensor_tensor(
                    out=cmb[:, 0:K, :], in0=vn_t, in1=z_t, op=Alu.mult
                )
                nc.scalar.activation(
                    out=cmb[:, K : 2 * K, :], in_=vn_t, func=Act.Square
                )
                hh = hpool.tile([P, 2 * K, 64], f16, tag="hh")
                nc.vector.tensor_tensor(
                    out=hh, in0=cmb[:, :, 0:64], in1=cmb[:, :, 64:128], op=Alu.add
                )
                hh2 = hpool.tile([P, 2 * K, 32], f16, tag="hh2")
                nc.vector.tensor_tensor(
                    out=hh2, in0=hh[:, :, 0:32], in1=hh[:, :, 32:64], op=Alu.add
                )
                nd_t = smpool.tile([P, 2 * K], f32, tag="nd")
                nc.vector.tensor_reduce(
                    out=nd_t, in_=hh2, axis=mybir.AxisListType.X, op=Alu.add
                )

                rn_t = smpool.tile([P, K], f32, tag="rn")
                nc.vector.reciprocal(out=rn_t, in_=nd_t[:, K : 2 * K])
                s_t = smpool.tile([P, K], f32, tag="s")
                nc.vector.scalar_tensor_tensor(
                    out=s_t,
                    in0=nd_t[:, 0:K],
                    scalar=-2.0,
                    in1=rn_t,
                    op0=Alu.mult,
                    op1=Alu.mult,
                )

                # t = vn * s via one Pool tensor_tensor with a stride-0
                # broadcast of s along features.
                t_t = vnpool.tile([P, K, D], f16, tag="t")
                s_bc = s_t.unsqueeze(2).broadcast_to([P, K, D])
                nc.gpsimd.tensor_tensor(out=t_t, in0=vn_t, in1=s_bc, op=Alu.mult)

                # Software-pipelined tail: while Pool computes t(g), DVE
                # finishes z_new(g-1) = t(g-1) + z(g-1) (fp16 2x) so the
                # in-order DVE queue never stalls on the Pool round-trip.
                if prev is not None:
                    pt, pz, pg = prev
                    zn_t = znpool.tile([P, K, D], f16, tag="zn")
                    nc.vector.tensor_tensor(out=zn_t, in0=pt, in1=pz, op=Alu.add)
                    nc.scalar.dma_start(out=ov[pg], in_=zn_t)
                prev = (t_t, z_t, g)

            pt, pz, pg = prev
            zn_t = znpool.tile([P, K, D], f16, tag="zn")
            nc.vector.tensor_tensor(out=zn_t, in0=pt, in1=pz, op=Alu.add)
            nc.scalar.dma_start(out=ov[pg], in_=zn_t)

    nc.compile()
    return nc


def _get_compiled():
    global _compiled
    if _compiled is None:
        _compiled = _build()
    return _compiled


def kernel(z, v, W, b):
    from concourse.bass_utils import run_bass_kernel_spmd

    nc = _get_compiled()

    z16 = np.ascontiguousarray(np.asarray(z), dtype=np.float16)
    v16 = np.asarray(v, dtype=np.float16)
    # vt[f, g*2048 + k*128 + p] = v[g*2048 + p*16 + k, f]
    vt = np.ascontiguousarray(
        v16.reshape(GT, P, K, D).transpose(3, 0, 2, 1).reshape(D, B)
    )
    wt = np.ascontiguousarray(np.asarray(W, dtype=np.float16).T)
    b4 = np.ascontiguousarray(
        np.tile(np.asarray(b, dtype=np.float16).reshape(1, D), (1, 4))
    )

    in_maps = []
    for c in range(NCORES):
        sl = slice(c * ROWS_PER_CORE, (c + 1) * ROWS_PER_CORE)
        in_maps.append(
            {
                "z": z16[sl],
                "vt": np.ascontiguousarray(vt[:, sl]),
                "wt": wt,
                "b4": b4,
            }
        )

    res = run_bass_kernel_spmd(
        nc, in_maps, core_ids=list(range(NCORES)), trace=TRACE, **TRACE_KW
    )
    global LAST
    LAST = res
    out16 = np.concatenate([res.results[c]["z_new"] for c in range(NCORES)], axis=0)
    return out16.astype(np.float32)



# revision 4
# speedup vs baseline: 1.0686x; 1.0414x over previous
"""Householder reflection kernel for Trainium2 (8 NeuronCores, data-parallel).

Computes: v_new = v @ W.T + b
          z_new = z - 2 * v_new * (v_new . z) / ||v_new||^2

Full inputs: z [524288, 128] f32, v [524288, 128] f32, W [128, 128] f32, b [128] f32.
Sharding: batch split 8 ways; W/b replicated.

Strategy (fp16 data movement, fp32 accumulation):
  - Host: cast z to fp16; build vt = v.T in fp16 with rows permuted so that
    both the z/out DMAs and the vt DMA are 4KB-contiguous per partition,
    and each matmul's stationary operand is a [feat, rows] chunk of vt
    (no on-chip transposes needed at all).
    Row mapping: global row r = g*2048 + p*16 + k  <->  (group g, partition p,
    chunk k); vt column order is g*2048 + k*128 + p.
  - PE: per 128-row chunk, psum[:,k,:] = vt_chunk.T @ W.T (fp16 in, fp32 acc),
    bias added via a rank-1 ones*b4 matmul per PSUM bank (4 chunks at once).
  - ACT: copy psum -> vn (fp16 sbuf) and Square psum -> sq (fp16 sbuf).
  - DVE: tree-reduce sq -> norm [128,16] f32; prod = vn*z (2x fp16 mode) and
    tree-reduce -> dot; s = -2*dot*recip(norm).
  - Pool (gpsimd): per-chunk fused z_new = vn*s + z.
"""

import sys

if "/opt/trn_rl_repo" not in sys.path:
    sys.path.insert(0, "/opt/trn_rl_repo")

import numpy as np

B = 524288
D = 128
NCORES = 8
ROWS_PER_CORE = B // NCORES          # 65536
P = 128                              # rows per chunk (psum partitions)
K = 16                               # chunks per group
ROWS_PER_GROUP = K * P               # 2048
GROUPS = ROWS_PER_CORE // ROWS_PER_GROUP  # 32
GT = B // ROWS_PER_GROUP             # 256 global groups

TRACE = False
TRACE_KW = {}
LAST = None
TSPLIT = 8  # chunks of t computed on Pool; rest on DVE tensor_scalar

_compiled = None


def _build():
    import concourse.bacc as bacc
    import concourse.tile as tile
    from concourse import mybir

    nc = bacc.Bacc("TRN2")
    f16 = mybir.dt.float16
    f32 = mybir.dt.float32
    Alu = mybir.AluOpType
    Act = mybir.ActivationFunctionType

    z_d = nc.dram_tensor("z", [ROWS_PER_CORE, D], f16, kind="ExternalInput")
    vt_d = nc.dram_tensor("vt", [D, ROWS_PER_CORE], f16, kind="ExternalInput")
    wt_d = nc.dram_tensor("wt", [D, D], f16, kind="ExternalInput")
    b4_d = nc.dram_tensor("b4", [1, 4 * D], f16, kind="ExternalInput")
    out_d = nc.dram_tensor("z_new", [ROWS_PER_CORE, D], f16, kind="ExternalOutput")

    # DRAM views; all give >=4KB-contiguous per-partition DMA segments.
    # Loads are paired two groups per DMA to halve DGE/semaphore overhead.
    zv2 = z_d.rearrange("(g t p k) f -> g p t k f", t=2, p=P, k=K)
    ov = out_d.rearrange("(g p k) f -> g p k f", p=P, k=K)
    vv2 = vt_d.rearrange("f (g t k p) -> g f t k p", t=2, k=K, p=P)

    with tile.TileContext(nc) as tc:
        from contextlib import ExitStack

        with ExitStack() as ctx:
            singles = ctx.enter_context(tc.tile_pool(name="singles", bufs=1))
            inpool = ctx.enter_context(tc.tile_pool(name="inp", bufs=3))
            vnpool = ctx.enter_context(tc.tile_pool(name="vn", bufs=5))
            sqpool = ctx.enter_context(tc.tile_pool(name="sq", bufs=4))
            hpool = ctx.enter_context(tc.tile_pool(name="h", bufs=4))
            smpool = ctx.enter_context(tc.tile_pool(name="sm", bufs=6))
            znpool = ctx.enter_context(tc.tile_pool(name="zn", bufs=4))
            ppool = ctx.enter_context(tc.tile_pool(name="ps", bufs=2, space="PSUM"))

            wt_sb = singles.tile([D, D], f16)
            nc.sync.dma_start(out=wt_sb, in_=wt_d.ap())
            b4_sb = singles.tile([1, 4 * D], f16)
            nc.sync.dma_start(out=b4_sb, in_=b4_d.ap())
            ones1 = singles.tile([1, D], f16)
            nc.vector.memset(ones1, 1.0)

            prev = None
            for g in range(GROUPS):
                if g % 2 == 0:
                    vt2 = inpool.tile([P, 2, K, P], f16, tag="vt")
                    z2 = inpool.tile([P, 2, K, D], f16, tag="z")
                    nc.sync.dma_start(out=vt2, in_=vv2[g // 2])
                    nc.sync.dma_start(out=z2, in_=zv2[g // 2])
                vt_t = vt2[:, g % 2]
                z_t = z2[:, g % 2]

                # Half-group PSUM tiles (2 banks, 2 bufs per tag): ACT drains
                # each half while PE fills the next, shortening the
                # PE(g+1) <- COPY(g) PSUM-reuse stall.
                vn_t = vnpool.tile([P, K, D], f16, tag="vn")
                H = K // 2
                for h, ptag in ((0, "psa"), (1, "psb")):
                    ps = ppool.tile([P, H, D], f32, tag=ptag)
                    for bb in range(H // 4):
                        nc.tensor.matmul(
                            ps[:, 4 * bb : 4 * bb + 4, :],
                            lhsT=ones1,
                            rhs=b4_sb,
                            start=True,
                            stop=False,
                            skip_group_check=True,
                        )
                        for kk in range(4 * bb, 4 * bb + 4):
                            k = h * H + kk
                            nc.tensor.matmul(
                                ps[:, kk, :],
                                lhsT=vt_t[:, k, :],
                                rhs=wt_sb,
                                start=False,
                                stop=True,
                                skip_group_check=True,
                            )
                    nc.scalar.copy(out=vn_t[:, h * H : (h + 1) * H, :], in_=ps)

                # Combined tile: [:, 0:K] = pr = vn*z (DVE), [:, K:2K] = vn^2
                # (ACT Square). One shared tree then reduces dot and norm
                # together: 3 DVE ops instead of 5 (per-op overhead is the
                # dominant cost at these sizes).
                cmb = sqpool.tile([P, 2 * K, D], f16, tag="cmb")
                nc.vector.tensor_tensor(
                    out=cmb[:, 0:K, :], in0=vn_t, in1=z_t, op=Alu.mult
                )
                nc.scalar.activation(
                    out=cmb[:, K : 2 * K, :], in_=vn_t, func=Act.Square
                )
                hh = hpool.tile([P, 2 * K, 64], f16, tag="hh")
                nc.vector.tensor_tensor(
                    out=hh, in0=cmb[:, :, 0:64], in1=cmb[:, :, 64:128], op=Alu.add
                )
                hh2 = hpool.tile([P, 2 * K, 32], f16, tag="hh2")
                nc.vector.tensor_tensor(
                    out=hh2, in0=hh[:, :, 0:32], in1=hh[:, :, 32:64], op=Alu.add
                )
                nd_t = smpool.tile([P, 2 * K], f32, tag="nd")
                nc.vector.tensor_reduce(
                    out=nd_t, in_=hh2, axis=mybir.AxisListType.X, op=Alu.add
                )

                rn_t = smpool.tile([P, K], f32, tag="rn")
                nc.vector.reciprocal(out=rn_t, in_=nd_t[:, K : 2 * K])
                s_t = smpool.tile([P, K], f32, tag="s")
                nc.vector.scalar_tensor_tensor(
                    out=s_t,
                    in0=nd_t[:, 0:K],
                    scalar=-2.0,
                    in1=rn_t,
                    op0=Alu.mult,
                    op1=Alu.mult,
                )

                # t = vn * s: split between Pool (stride-0 broadcast mult)
                # and DVE per-chunk tensor_scalar (per-partition scalar AP).
                # Pool's SBUF port is DVE's rd1, so every concurrent 2-input
                # DVE op runs ~2x slower while Pool is busy; shrinking the
                # Pool op trades that tax against cheap 1-port DVE ts-ops.
                t_t = vnpool.tile([P, K, D], f16, tag="t")
                XP = TSPLIT  # chunks on Pool
                if XP > 0:
                    s_bc = (
                        s_t[:, 0:XP].unsqueeze(2).broadcast_to([P, XP, D])
                    )
                    nc.gpsimd.tensor_tensor(
                        out=t_t[:, 0:XP, :], in0=vn_t[:, 0:XP, :], in1=s_bc, op=Alu.mult
                    )
                for k in range(XP, K):
                    nc.vector.tensor_scalar(
                        out=t_t[:, k, :],
                        in0=vn_t[:, k, :],
                        scalar1=s_t[:, k : k + 1],
                        scalar2=None,
                        op0=Alu.mult,
                    )

                # Software-pipelined tail: while Pool computes t(g), DVE
                # finishes z_new(g-1) = t(g-1) + z(g-1) (fp16 2x) so the
                # in-order DVE queue never stalls on the Pool round-trip.
                if prev is not None:
                    pt, pz, pg = prev
                    zn_t = znpool.tile([P, K, D], f16, tag="zn")
                    nc.vector.tensor_tensor(out=zn_t, in0=pt, in1=pz, op=Alu.add)
                    nc.scalar.dma_start(out=ov[pg], in_=zn_t)
                prev = (t_t, z_t, g)

            pt, pz, pg = prev
            zn_t = znpool.tile([P, K, D], f16, tag="zn")
            nc.vector.tensor_tensor(out=zn_t, in0=pt, in1=pz, op=Alu.add)
            nc.scalar.dma_start(out=ov[pg], in_=zn_t)

    nc.compile()
    return nc


def _get_compiled():
    global _compiled
    if _compiled is None:
        _compiled = _build()
    return _compiled


def kernel(z, v, W, b):
    from concourse.bass_utils import run_bass_kernel_spmd

    nc = _get_compiled()

    z16 = np.ascontiguousarray(np.asarray(z), dtype=np.float16)
    v16 = np.asarray(v, dtype=np.float16)
    # vt[f, g*2048 + k*128 + p] = v[g*2048 + p*16 + k, f]
    vt = np.ascontiguousarray(
        v16.reshape(GT, P, K, D).transpose(3, 0, 2, 1).reshape(D, B)
    )
    wt = np.ascontiguousarray(np.asarray(W, dtype=np.float16).T)
    b4 = np.ascontiguousarray(
        np.tile(np.asarray(b, dtype=np.float16).reshape(1, D), (1, 4))
    )

    in_maps = []
    for c in range(NCORES):
        sl = slice(c * ROWS_PER_CORE, (c + 1) * ROWS_PER_CORE)
        in_maps.append(
            {
                "z": z16[sl],
                "vt": np.ascontiguousarray(vt[:, sl]),
                "wt": wt,
                "b4": b4,
            }
        )

    res = run_bass_kernel_spmd(
        nc, in_maps, core_ids=list(range(NCORES)), trace=TRACE, **TRACE_KW
    )
    global LAST
    LAST = res
    out16 = np.concatenate([res.results[c]["z_new"] for c in range(NCORES)], axis=0)
    return out16.astype(np.float32)



# revision 5
# speedup vs baseline: 1.3239x; 1.2389x over previous
"""Householder reflection kernel for Trainium2 (8 NeuronCores, data-parallel).

Computes: v_new = v @ W.T + b
          z_new = z - 2 * v_new * (v_new . z) / ||v_new||^2

Full inputs: z [524288, 128] f32, v [524288, 128] f32, W [128, 128] f32, b [128] f32.
Sharding: batch split 8 ways; W/b replicated.

Strategy (fp16 data movement, fp32 accumulation):
  - Host: cast z to fp16; build vt = v.T in fp16 with rows permuted so that
    both the z/out DMAs and the vt DMA are 4KB-contiguous per partition,
    and each matmul's stationary operand is a [feat, rows] chunk of vt
    (no on-chip transposes needed at all).
    Row mapping: global row r = g*2048 + p*16 + k  <->  (group g, partition p,
    chunk k); vt column order is g*2048 + k*128 + p.
  - PE: per 128-row chunk, psum[:,k,:] = vt_chunk.T @ W.T (fp16 in, fp32 acc),
    bias added via a rank-1 ones*b4 matmul per PSUM bank (4 chunks at once).
  - ACT: copy psum -> vn (fp16 sbuf) and Square psum -> sq (fp16 sbuf).
  - DVE: tree-reduce sq -> norm [128,16] f32; prod = vn*z (2x fp16 mode) and
    tree-reduce -> dot; s = -2*dot*recip(norm).
  - Pool (gpsimd): per-chunk fused z_new = vn*s + z.
"""

import sys

if "/opt/trn_rl_repo" not in sys.path:
    sys.path.insert(0, "/opt/trn_rl_repo")

import numpy as np

B = 524288
D = 128
NCORES = 8
ROWS_PER_CORE = B // NCORES          # 65536
P = 128                              # rows per chunk (psum partitions)
K = 16                               # chunks per group
ROWS_PER_GROUP = K * P               # 2048
GROUPS = ROWS_PER_CORE // ROWS_PER_GROUP  # 32
GT = B // ROWS_PER_GROUP             # 256 global groups

TRACE = False
TRACE_KW = {}
LAST = None
TSPLIT = 8  # chunks of t computed on Pool; rest on DVE tensor_scalar

_compiled = None


def _build():
    import concourse.bacc as bacc
    import concourse.tile as tile
    from concourse import mybir

    nc = bacc.Bacc("TRN2")
    f16 = mybir.dt.float16
    f32 = mybir.dt.float32
    Alu = mybir.AluOpType
    Act = mybir.ActivationFunctionType

    z_d = nc.dram_tensor("z", [ROWS_PER_CORE, D], f16, kind="ExternalInput")
    vt_d = nc.dram_tensor("vt", [D, ROWS_PER_CORE], f16, kind="ExternalInput")
    wt_d = nc.dram_tensor("wt", [D, D], f16, kind="ExternalInput")
    b4_d = nc.dram_tensor("b4", [1, 4 * D], f16, kind="ExternalInput")
    out_d = nc.dram_tensor("z_new", [ROWS_PER_CORE, D], f16, kind="ExternalOutput")

    # DRAM views; all give >=4KB-contiguous per-partition DMA segments.
    # Loads are paired two groups per DMA to halve DGE/semaphore overhead.
    zv2 = z_d.rearrange("(g t p k) f -> g p t k f", t=2, p=P, k=K)
    ov = out_d.rearrange("(g p k) f -> g p k f", p=P, k=K)
    vv2 = vt_d.rearrange("f (g t k p) -> g f t k p", t=2, k=K, p=P)

    with tile.TileContext(nc) as tc:
        from contextlib import ExitStack

        with ExitStack() as ctx:
            singles = ctx.enter_context(tc.tile_pool(name="singles", bufs=1))
            inpool = ctx.enter_context(tc.tile_pool(name="inp", bufs=3))
            vnpool = ctx.enter_context(tc.tile_pool(name="vn", bufs=5))
            sqpool = ctx.enter_context(tc.tile_pool(name="sq", bufs=4))
            hpool = ctx.enter_context(tc.tile_pool(name="h", bufs=4))
            smpool = ctx.enter_context(tc.tile_pool(name="sm", bufs=6))
            znpool = ctx.enter_context(tc.tile_pool(name="zn", bufs=4))
            ppool = ctx.enter_context(tc.tile_pool(name="ps", bufs=2, space="PSUM"))

            wt_sb = singles.tile([D, D], f16)
            nc.sync.dma_start(out=wt_sb, in_=wt_d.ap())
            b4_sb = singles.tile([1, 4 * D], f16)
            nc.sync.dma_start(out=b4_sb, in_=b4_d.ap())
            ones1 = singles.tile([1, D], f16)
            nc.vector.memset(ones1, 1.0)

            prev = None
            for g in range(GROUPS):
                if g % 2 == 0:
                    vt2 = inpool.tile([P, 2, K, P], f16, tag="vt")
                    z2 = inpool.tile([P, 2, K, D], f16, tag="z")
                    nc.sync.dma_start(out=vt2, in_=vv2[g // 2])
                    nc.sync.dma_start(out=z2, in_=zv2[g // 2])
                vt_t = vt2[:, g % 2]
                z_t = z2[:, g % 2]

                # Half-group PSUM tiles (2 banks, 2 bufs per tag): ACT drains
                # each half while PE fills the next, shortening the
                # PE(g+1) <- COPY(g) PSUM-reuse stall.
                vn_t = vnpool.tile([P, K, D], f16, tag="vn")
                H = K // 2
                for h, ptag in ((0, "psa"), (1, "psb")):
                    ps = ppool.tile([P, H, D], f32, tag=ptag)
                    for bb in range(H // 4):
                        nc.tensor.matmul(
                            ps[:, 4 * bb : 4 * bb + 4, :],
                            lhsT=ones1,
                            rhs=b4_sb,
                            start=True,
                            stop=False,
                            skip_group_check=True,
                        )
                        for kk in range(4 * bb, 4 * bb + 4):
                            k = h * H + kk
                            nc.tensor.matmul(
                                ps[:, kk, :],
                                lhsT=vt_t[:, k, :],
                                rhs=wt_sb,
                                start=False,
                                stop=True,
                                skip_group_check=True,
                            )
                    nc.scalar.copy(out=vn_t[:, h * H : (h + 1) * H, :], in_=ps)

                # Combined tile: [:, 0:K] = pr = vn*z (DVE), [:, K:2K] = vn^2
                # (ACT Square). One shared tree then reduces dot and norm
                # together: 3 DVE ops instead of 5 (per-op overhead is the
                # dominant cost at these sizes).
                cmb = sqpool.tile([P, 2 * K, D], f16, tag="cmb")
                nc.vector.tensor_tensor(
                    out=cmb[:, 0:K, :], in0=vn_t, in1=z_t, op=Alu.mult
                )
                nc.scalar.activation(
                    out=cmb[:, K : 2 * K, :], in_=vn_t, func=Act.Square
                )
                hh = hpool.tile([P, 2 * K, 64], f16, tag="hh")
                nc.vector.tensor_tensor(
                    out=hh, in0=cmb[:, :, 0:64], in1=cmb[:, :, 64:128], op=Alu.add
                )
                hh2 = hpool.tile([P, 2 * K, 32], f16, tag="hh2")
                nc.vector.tensor_tensor(
                    out=hh2, in0=hh[:, :, 0:32], in1=hh[:, :, 32:64], op=Alu.add
                )
                nd_t = smpool.tile([P, 2 * K], f32, tag="nd")
                nc.vector.tensor_reduce(
                    out=nd_t, in_=hh2, axis=mybir.AxisListType.X, op=Alu.add
                )

                rn_t = smpool.tile([P, K], f32, tag="rn")
                nc.vector.reciprocal(out=rn_t, in_=nd_t[:, K : 2 * K])
                s_t = smpool.tile([P, K], f32, tag="s")
                nc.vector.scalar_tensor_tensor(
                    out=s_t,
                    in0=nd_t[:, 0:K],
                    scalar=-2.0,
                    in1=rn_t,
                    op0=Alu.mult,
                    op1=Alu.mult,
                )

                # t = vn * s as ONE DVE tensor_tensor at 2x: duplicate each s
                # into an fp16 pair (s16 [P,K,2]) so the broadcast AP's
                # innermost dim is [stride 1, count 2] -- satisfying the
                # 2x_1p packing rule that a plain stride-0 broadcast misses.
                # Pool is intentionally unused: its SBUF port is DVE's rd1,
                # and any concurrent Pool op halves DVE 2-port throughput.
                s16 = smpool.tile([P, K, 2], f16, tag="s16")
                nc.vector.tensor_scalar(
                    out=s16,
                    in0=s_t.unsqueeze(2).broadcast_to([P, K, 2]),
                    scalar1=1.0,
                    scalar2=None,
                    op0=Alu.mult,
                )
                t_t = vnpool.tile([P, K, D], f16, tag="t")
                s_bc = s16.unsqueeze(2).broadcast_to([P, K, 64, 2])
                nc.vector.tensor_tensor(
                    out=t_t.rearrange("p k (f two) -> p k f two", two=2),
                    in0=vn_t.rearrange("p k (f two) -> p k f two", two=2),
                    in1=s_bc,
                    op=Alu.mult,
                )

                # Software-pipelined tail: while Pool computes t(g), DVE
                # finishes z_new(g-1) = t(g-1) + z(g-1) (fp16 2x) so the
                # in-order DVE queue never stalls on the Pool round-trip.
                if prev is not None:
                    pt, pz, pg = prev
                    zn_t = znpool.tile([P, K, D], f16, tag="zn")
                    nc.vector.tensor_tensor(out=zn_t, in0=pt, in1=pz, op=Alu.add)
                    nc.scalar.dma_start(out=ov[pg], in_=zn_t)
                prev = (t_t, z_t, g)

            pt, pz, pg = prev
            zn_t = znpool.tile([P, K, D], f16, tag="zn")
            nc.vector.tensor_tensor(out=zn_t, in0=pt, in1=pz, op=Alu.add)
            nc.scalar.dma_start(out=ov[pg], in_=zn_t)

    nc.compile()
    return nc


def _get_compiled():
    global _compiled
    if _compiled is None:
        _compiled = _build()
    return _compiled


def kernel(z, v, W, b):
    from concourse.bass_utils import run_bass_kernel_spmd

    nc = _get_compiled()

    z16 = np.ascontiguousarray(np.asarray(z), dtype=np.float16)
    v16 = np.asarray(v, dtype=np.float16)
    # vt[f, g*2048 + k*128 + p] = v[g*2048 + p*16 + k, f]
    vt = np.ascontiguousarray(
        v16.reshape(GT, P, K, D).transpose(3, 0, 2, 1).reshape(D, B)
    )
    wt = np.ascontiguousarray(np.asarray(W, dtype=np.float16).T)
    b4 = np.ascontiguousarray(
        np.tile(np.asarray(b, dtype=np.float16).reshape(1, D), (1, 4))
    )

    in_maps = []
    for c in range(NCORES):
        sl = slice(c * ROWS_PER_CORE, (c + 1) * ROWS_PER_CORE)
        in_maps.append(
            {
                "z": z16[sl],
                "vt": np.ascontiguousarray(vt[:, sl]),
                "wt": wt,
                "b4": b4,
            }
        )

    res = run_bass_kernel_spmd(
        nc, in_maps, core_ids=list(range(NCORES)), trace=TRACE, **TRACE_KW
    )
    global LAST
    LAST = res
    out16 = np.concatenate([res.results[c]["z_new"] for c in range(NCORES)], axis=0)
    return out16.astype(np.float32)

